# revision 1
# baseline (speedup 1.0000x reference)
"""AFT transformer block on 8 Trainium2 NeuronCores.

Data-parallel over batch: each core runs the full block for 4 of the 32
sequences (the AFT attention mixes only within a sequence, so no
collectives are needed).  Host side folds the shared LayerNorm affine
params into the GEMM weights, precomputes exp(wbias).T, and quantizes
the QKV/output-projection and FFN down-projection weights to fp8-e4m3;
those GEMMs run in DoubleRow perf mode (two contraction tiles per
matmul, 2x tensor-engine throughput).  The positional-bias GEMMs and
the FFN up-projection stay bf16 for accuracy.

Scaling scheme (keeps every fp8 operand in e4m3's normal range with no
extra de-scale ops on the device):
  x is pre-scaled by SR=16 on the host, so the residual stream is 16x.
  LayerNorm output h is stored at SH=1/4 scale, QKV/proj weights at
  SW=4, so all QKV psums are exact-scale.  Y^T is stored at SY=4
  (folded into e1=(1+e)/SY), wp at 4 -> proj psum lands at 16x,
  matching the residual.  FFN: h2 unit-scale bf16, w1 unscaled bf16,
  gelu output G unscaled fp8, w2 at SW2=16 -> down psum at 16x.  The
  host divides the output by 16.
"""

import numpy as np
import ml_dtypes

import concourse.bass as bass
import concourse.mybir as mybir
import concourse.tile as tile
from concourse import bacc
from concourse.bass_utils import run_bass_kernel_spmd
from concourse.masks import make_identity

F32 = mybir.dt.float32
U32 = mybir.dt.uint32
BF16 = mybir.dt.bfloat16
F8 = mybir.dt.float8e4
AF = mybir.ActivationFunctionType
ALU = mybir.AluOpType
DRM = mybir.MatmulPerfMode.DoubleRow

B, T, D, FF = 32, 512, 1024, 4096
NCORES = 8
NB = B // NCORES          # sequences per core (4)
NT = NB * T               # tokens per core (2048)
KD = D // 128             # 8
KD2 = KD // 2             # 4 doublerow pairs over D
KF = FF // 128            # 32
KF2 = KF // 2             # 16 doublerow pairs over FF
TT = NT // 128            # 16 token tiles per core
EPS = 1e-5

SR = 16.0                 # residual-stream scale (x pre-scaled on host)
SH = 0.25                 # stored LN1-output scale
SW = 4.0                  # qkv/proj weight scale
SY = 4.0                  # stored Y^T scale
SW2 = 16.0                # w2 weight scale
EPS_S = EPS * SR * SR


def build_nc(with_bias: bool):
    nc = bacc.Bacc("TRN2", target_bir_lowering=False, debug=False, num_devices=NCORES)

    x_ext = nc.dram_tensor("x", [NT, D], F32, kind="ExternalInput").ap()
    # fp8 doublerow-paired weights: [pair, partition(k-within), 2, cols]
    wqp_ext = nc.dram_tensor("wqp", [KD2, 128, 2, D], F8, kind="ExternalInput").ap()
    wkp_ext = nc.dram_tensor("wkp", [KD2, 128, 2, D], F8, kind="ExternalInput").ap()
    wvp_ext = nc.dram_tensor("wvp", [KD2, 128, 2, D], F8, kind="ExternalInput").ap()
    wpp_ext = nc.dram_tensor("wpp", [KD2, 128, 2, D], F8, kind="ExternalInput").ap()
    # w1 pre-tiled on host (bf16): w1f[f, p, k*128+c] = w1[k*128+p, f*128+c]
    w1f_ext = nc.dram_tensor("w1f", [KF, 128, D], BF16, kind="ExternalInput").ap()
    w2p_ext = nc.dram_tensor("w2p", [KF2, 128, 2, D], F8, kind="ExternalInput").ap()
    wbT_ext = nc.dram_tensor("wbT", [T, T], BF16, kind="ExternalInput").ap()
    # rows: 0=bk', 1=bv', 2=bp'*SR, 3=b2'*SR (rank-1 bias matmul operands)
    brow_ext = nc.dram_tensor("brow", [4, D], BF16, kind="ExternalInput").ap()
    bqcol_ext = nc.dram_tensor("bqcol", [128, KD], F32, kind="ExternalInput").ap()
    b1col_ext = nc.dram_tensor("b1col", [128, KF], F32, kind="ExternalInput").ap()
    out_ext = nc.dram_tensor("out", [NT, D], F32, kind="ExternalOutput").ap()

    with tile.TileContext(nc) as tc:
        with (
            tc.tile_pool(name="const", bufs=1) as constp,
            tc.tile_pool(name="dstats", bufs=1) as dstats,
            tc.tile_pool(name="ps_t", bufs=2, space="PSUM") as ps_t,
            tc.tile_pool(name="x1dp", bufs=1, space="DRAM") as x1dp,
        ):
            ident = constp.tile([128, 128], BF16, tag="ident")
            make_identity(nc, ident)
            ones1 = constp.tile([1, 128], BF16, tag="ones1")
            nc.vector.memset(ones1, 1.0)
            wbT_sb = constp.tile([128, 4, T], BF16, tag="wbT")
            brows = []
            for r in range(4):
                br = constp.tile([1, D], BF16, tag=f"brow{r}", name=f"brow{r}")
                nc.sync.dma_start(out=br, in_=brow_ext[r:r + 1, :])
                brows.append(br)
            bqcol = constp.tile([128, KD], F32, tag="bqcol")
            nc.sync.dma_start(out=bqcol, in_=bqcol_ext[:, :])
            b1col = constp.tile([128, KF], F32, tag="b1col")
            nc.sync.dma_start(out=b1col, in_=b1col_ext[:, :])
            # warm the exp table set while the first activation DMAs land
            wt = constp.tile([128, 1], F32, tag="wt")
            nc.vector.memset(wt, 0.0)
            nc.scalar.activation(out=wt, in_=wt, func=AF.Exp)

            # per-token-tile LayerNorm stats, [128, tt] (one column per tile)
            mean1 = dstats.tile([128, TT], F32, tag="mean1")
            var1 = dstats.tile([128, TT], F32, tag="var1")
            rstd1 = dstats.tile([128, TT], F32, tag="rstd1")
            nmr1 = dstats.tile([128, TT], F32, tag="nmr1")
            mean2 = dstats.tile([128, TT], F32, tag="mean2")
            var2 = dstats.tile([128, TT], F32, tag="var2")
            rstd2 = dstats.tile([128, TT], F32, tag="rstd2")
            nmr2 = dstats.tile([128, TT], F32, tag="nmr2")
            nve = dstats.tile([128, 4], F32, tag="nve")   # rsqrt Newton scratch
            nt = dstats.tile([128, 4], F32, tag="nt")

            def rsqrt_dve(y, nmr_ap, mean_ap, var_ap, n, out_scale):
                """y = out_scale/sqrt(var+EPS_S), nmr = -mean*y (DVE only).
                y/nmr_ap/mean_ap/var_ap are pre-sliced [128, n] APs."""
                ve = nve[:, 0:n]
                t = nt[:, 0:n]
                nc.vector.tensor_scalar_add(out=ve, in0=var_ap, scalar1=EPS_S)
                nc.vector.tensor_scalar(out=y.bitcast(U32), in0=ve.bitcast(U32),
                                        scalar1=1, scalar2=None,
                                        op0=ALU.logical_shift_right)
                nc.vector.tensor_scalar(out=y.bitcast(U32), in0=y.bitcast(U32),
                                        scalar1=0x5f3759df, scalar2=-1,
                                        op0=ALU.subtract, op1=ALU.mult)
                for it in range(3):
                    s = out_scale if it == 2 else 1.0
                    nc.vector.tensor_tensor(out=t, in0=ve, in1=y, op=ALU.mult)
                    nc.vector.tensor_tensor(out=t, in0=t, in1=y, op=ALU.mult)
                    nc.vector.tensor_scalar(out=t, in0=t, scalar1=-0.5 * s,
                                            scalar2=1.5 * s,
                                            op0=ALU.mult, op1=ALU.add)
                    nc.vector.tensor_tensor(out=y, in0=y, in1=t, op=ALU.mult)
                nc.vector.tensor_tensor(out=nmr_ap, in0=mean_ap, in1=y, op=ALU.mult)
                nc.vector.tensor_scalar_mul(out=nmr_ap, in0=nmr_ap, scalar1=-1.0)

            # DRAM scratch for the post-attention residual stream x1 (16x)
            x1d = [x1dp.tile([128, D], F32, tag=f"x1d{t}", name=f"x1d{t}") for t in range(TT)]

            # ---------------- attention sub-block ----------------
            with (
                tc.tile_pool(name="aw", bufs=1) as aw,
                tc.tile_pool(name="xbp", bufs=2) as xbp,
                tc.tile_pool(name="ps_mm", bufs=6, space="PSUM") as ps_mm,
            ):
                xb0 = [xbp.tile([128, D], F32, tag=f"xb{i}", name=f"xb{i}") for i in range(4)]
                for i in range(4):
                    nc.sync.dma_start(out=xb0[i], in_=x_ext[i * 128:(i + 1) * 128, :])

                def ln_stats(xt, mean_t, var_t, t, stats_pool):
                    st = stats_pool.tile([128, 2, 6], F32, tag="bnst", name="st")
                    nc.vector.bn_stats(out=st[:, 0, :], in_=xt[:, 0:512])
                    nc.vector.bn_stats(out=st[:, 1, :], in_=xt[:, 512:1024])
                    mvt = stats_pool.tile([128, 2], F32, tag="mvt", name="mvt")
                    nc.vector.bn_aggr(out=mvt, in_=st)
                    nc.gpsimd.tensor_copy(out=mean_t[:, t:t + 1], in_=mvt[:, 0:1])
                    nc.gpsimd.tensor_copy(out=var_t[:, t:t + 1], in_=mvt[:, 1:2])
                    return mvt

                wqp_sb = [aw.tile([128, 2, D], F8, tag=f"wq{k}", name=f"wq{k}") for k in range(KD2)]
                wkp_sb = [aw.tile([128, 2, D], F8, tag=f"wk{k}", name=f"wk{k}") for k in range(KD2)]
                wvp_sb = [aw.tile([128, 2, D], F8, tag=f"wv{k}", name=f"wv{k}") for k in range(KD2)]
                wpp_sb = [aw.tile([128, 2, D], F8, tag=f"wp{k}", name=f"wp{k}") for k in range(KD2)]
                for k in range(KD2):
                    nc.sync.dma_start(out=wqp_sb[k], in_=wqp_ext[k])
                for k in range(KD2):
                    nc.sync.dma_start(out=wkp_sb[k], in_=wkp_ext[k])
                    nc.sync.dma_start(out=wvp_sb[k], in_=wvp_ext[k])
                for s in range(4):
                    nc.sync.dma_start(out=wbT_sb[:, s, :], in_=wbT_ext[s * 128:(s + 1) * 128, :])
                for k in range(KD2):
                    nc.sync.dma_start(out=wpp_sb[k], in_=wpp_ext[k])

                with (
                    tc.tile_pool(name="ab", bufs=1) as ab,
                    tc.tile_pool(name="tmp", bufs=2) as tmp,
                    tc.tile_pool(name="sip", bufs=3) as sip,
                ):
                    # tile 0 fast path: rsqrt straight from bn_aggr output,
                    # skipping the gpsimd scatter hop (shorter startup chain)
                    mvt0 = ln_stats(xb0[0], mean1, var1, 0, sip)
                    rsqrt_dve(rstd1[:, 0:1], nmr1[:, 0:1],
                              mvt0[:, 0:1], mvt0[:, 1:2], 1, SH)
                    for t in range(1, 4):
                        ln_stats(xb0[t], mean1, var1, t, sip)
                    rsqrt_dve(rstd1[:, 1:4], nmr1[:, 1:4],
                              mean1[:, 1:4], var1[:, 1:4], 3, SH)

                    def norm_transpose(xb_t, col, h0T_t, i):
                        """h0 = SH*LN(x) (bf16) -> PE transpose -> fp8 pairs."""
                        h0 = tmp.tile([128, D], BF16, tag="h0", bufs=3)
                        nc.scalar.activation(out=h0, in_=xb_t, func=AF.Identity,
                                             bias=nmr1[:, col:col + 1],
                                             scale=rstd1[:, col:col + 1])
                        for k in range(KD):
                            tp = ps_t.tile([128, 128], BF16, tag="tp")
                            nc.tensor.transpose(tp, h0[:, k * 128:(k + 1) * 128], ident)
                            nc.vector.tensor_copy(
                                out=h0T_t[k // 2][:, k % 2, i * 128:(i + 1) * 128], in_=tp)

                    xb = xb0
                    x1_prev = None
                    # batch-0 transposes emitted ahead of the loop
                    h0T = [ab.tile([128, 2, T], F8, tag=f"h0T{k}", bufs=2, name=f"h0T{k}")
                           for k in range(KD2)]
                    for i in range(4):
                        norm_transpose(xb0[i], i, h0T, i)
                    for b in range(NB):
                        if b + 1 < NB:
                            xb_next = [xbp.tile([128, D], F32, tag=f"xb{i}", name=f"xb{i}")
                                       for i in range(4)]
                            for i in range(4):
                                t = (b + 1) * 4 + i
                                nc.sync.dma_start(out=xb_next[i],
                                                  in_=x_ext[t * 128:(t + 1) * 128, :])
                        eK = [ab.tile([128, D], BF16, tag=f"eK{i}", bufs=1, name=f"eK{i}")
                              for i in range(4)]
                        EV = [ab.tile([128, D], BF16, tag=f"EV{i}", bufs=1, name=f"EV{i}")
                              for i in range(4)]
                        eQ = [ab.tile([128, T], BF16, tag=f"eQ{j}", bufs=1, name=f"eQ{j}")
                              for j in range(KD)]
                        e1 = [ab.tile([128, T], BF16, tag=f"e1{j}", bufs=1, name=f"e1{j}")
                              for j in range(KD)]
                        YT = [ab.tile([128, 2, T], F8, tag=f"YT{j}", bufs=1, name=f"YT{j}")
                              for j in range(KD2)]
                        x1t = [ab.tile([128, D], F32, tag=f"x1t{i}", bufs=2, name=f"x1t{i}")
                               for i in range(4)]

                        # Q -> e = exp(Q + bq)  (sigmoid deferred into Y)
                        for j in range(KD):
                            qps = ps_mm.tile([128, T], F32, tag="mm")
                            for k in range(KD2):
                                nc.tensor.matmul(qps,
                                                 lhsT=wqp_sb[k][:, :, j * 128:(j + 1) * 128],
                                                 rhs=h0T[k], perf_mode=DRM,
                                                 start=(k == 0), stop=(k == KD2 - 1))
                            nc.scalar.activation(out=eQ[j], in_=qps, func=AF.Exp,
                                                 bias=bqcol[:, j:j + 1])
                            # e1 = (e + 1)/SY  (folds the Y^T storage scale)
                            nc.vector.tensor_scalar(out=e1[j], in0=eQ[j],
                                                    scalar1=1.0 / SY, scalar2=1.0 / SY,
                                                    op0=ALU.mult, op1=ALU.add)
                            # previous batch's LN2 stats ride the Q section's
                            # DVE slack (keeps the batch boundary clear)
                            if x1_prev is not None and j < 4:
                                ln_stats(x1_prev[j], mean2, var2,
                                         (b - 1) * 4 + j, sip)
                        if x1_prev is not None:
                            lo = (b - 1) * 4
                            rsqrt_dve(rstd2[:, lo:lo + 4], nmr2[:, lo:lo + 4],
                                      mean2[:, lo:lo + 4], var2[:, lo:lo + 4],
                                      4, 1.0)

                        # K, V (token-major) -> exp(K), exp(K)*V
                        for i in range(4):
                            for h in range(2):
                                sl = slice(h * 512, (h + 1) * 512)
                                kps = ps_mm.tile([128, 512], F32, tag="mm")
                                for k in range(KD2):
                                    nc.tensor.matmul(kps,
                                                     lhsT=h0T[k][:, :, i * 128:(i + 1) * 128],
                                                     rhs=wkp_sb[k][:, :, sl], perf_mode=DRM,
                                                     start=(k == 0),
                                                     stop=(not with_bias and k == KD2 - 1))
                                if with_bias:
                                    nc.tensor.matmul(kps, lhsT=ones1, rhs=brows[0][:, sl],
                                                     start=False, stop=True,
                                                     skip_group_check=True)
                                nc.scalar.activation(out=eK[i][:, sl], in_=kps, func=AF.Exp)
                                vps = ps_mm.tile([128, 512], F32, tag="mm")
                                for k in range(KD2):
                                    nc.tensor.matmul(vps,
                                                     lhsT=h0T[k][:, :, i * 128:(i + 1) * 128],
                                                     rhs=wvp_sb[k][:, :, sl], perf_mode=DRM,
                                                     start=(k == 0),
                                                     stop=(not with_bias and k == KD2 - 1))
                                if with_bias:
                                    nc.tensor.matmul(vps, lhsT=ones1, rhs=brows[1][:, sl],
                                                     start=False, stop=True,
                                                     skip_group_check=True)
                                nc.vector.tensor_tensor(out=EV[i][:, sl], in0=eK[i][:, sl],
                                                        in1=vps, op=ALU.mult)
                            if b + 1 < NB:
                                ln_stats(xb_next[i], mean1, var1, (b + 1) * 4 + i, sip)
                        if b + 1 < NB:
                            lo = (b + 1) * 4
                            rsqrt_dve(rstd1[:, lo:lo + 4], nmr1[:, lo:lo + 4],
                                      mean1[:, lo:lo + 4], var1[:, lo:lo + 4], 4, SH)

                        # positional-bias matmuls (feature-major, bf16) + Y epilogue
                        # YT = SY * num * e / ((1+e) * den)
                        for j in range(KD):
                            jsl = slice(j * 128, (j + 1) * 128)
                            dps = ps_mm.tile([128, T], F32, tag="mm")
                            for s in range(4):
                                nc.tensor.matmul(dps, lhsT=eK[s][:, jsl], rhs=wbT_sb[:, s, :],
                                                 start=(s == 0), stop=(s == 3))
                            nps = ps_mm.tile([128, T], F32, tag="mm")
                            for s in range(4):
                                nc.tensor.matmul(nps, lhsT=EV[s][:, jsl], rhs=wbT_sb[:, s, :],
                                                 start=(s == 0), stop=(s == 3))
                            dd = tmp.tile([128, T], F32, tag="dd")
                            nc.vector.tensor_tensor(out=dd, in0=dps, in1=e1[j], op=ALU.mult)
                            rd = tmp.tile([128, T], F32, tag="rd")
                            nc.vector.reciprocal_approx_fast(out=rd, in_=dd)
                            t1 = tmp.tile([128, T], F32, tag="t1")
                            nc.vector.tensor_tensor(out=t1, in0=nps, in1=rd, op=ALU.mult)
                            nc.vector.tensor_tensor(out=YT[j // 2][:, j % 2, :], in0=t1,
                                                    in1=eQ[j], op=ALU.mult)

                        # hoist the NEXT batch's normalize+transposes ahead of
                        # the projection so neither engine queue blocks at the
                        # batch boundary
                        if b + 1 < NB:
                            h0T_next = [ab.tile([128, 2, T], F8, tag=f"h0T{k}",
                                                bufs=2, name=f"h0T{k}")
                                        for k in range(KD2)]
                            for i in range(4):
                                norm_transpose(xb_next[i], (b + 1) * 4 + i,
                                               h0T_next, i)

                        # output projection (fp8 DR) + residual -> x1, LN2 stats
                        for i in range(4):
                            t = b * 4 + i
                            for h in range(2):
                                sl = slice(h * 512, (h + 1) * 512)
                                pps = ps_mm.tile([128, 512], F32, tag="mm")
                                for j in range(KD2):
                                    nc.tensor.matmul(pps,
                                                     lhsT=YT[j][:, :, i * 128:(i + 1) * 128],
                                                     rhs=wpp_sb[j][:, :, sl], perf_mode=DRM,
                                                     start=(j == 0),
                                                     stop=(not with_bias and j == KD2 - 1))
                                if with_bias:
                                    nc.tensor.matmul(pps, lhsT=ones1, rhs=brows[2][:, sl],
                                                     start=False, stop=True,
                                                     skip_group_check=True)
                                nc.vector.tensor_tensor(out=x1t[i][:, sl], in0=pps,
                                                        in1=xb[i][:, sl], op=ALU.add)
                            nc.sync.dma_start(out=x1d[t], in_=x1t[i])

                        if b == NB - 1:
                            # last batch: nobody left to defer to
                            for i in range(4):
                                ln_stats(x1t[i], mean2, var2, b * 4 + i, sip)
                            lo = b * 4
                            rsqrt_dve(rstd2[:, lo:lo + 4], nmr2[:, lo:lo + 4],
                                      mean2[:, lo:lo + 4], var2[:, lo:lo + 4],
                                      4, 1.0)
                        x1_prev = x1t
                        if b + 1 < NB:
                            xb = xb_next
                            h0T = h0T_next

            # ---------------- FFN sub-block ----------------
            # up-projection bf16 (precision), down-projection fp8 doublerow.
            with (
                tc.tile_pool(name="fw", bufs=1) as fw,
                tc.tile_pool(name="w1s", bufs=4) as w1s,
                tc.tile_pool(name="fb", bufs=1) as fb,
                tc.tile_pool(name="ftmp", bufs=3) as ftmp,
                tc.tile_pool(name="ps_o", bufs=3, space="PSUM") as ps_o,
                tc.tile_pool(name="ps_h1", bufs=2, space="PSUM") as ps_h1,
            ):
                w2p_sb = [fw.tile([128, 2, D], F8, tag=f"w2_{f}", name=f"w2_{f}")
                          for f in range(KF2)]

                def ffn_chunk_in(cc):
                    """Load + normalize + transpose one 512-token chunk."""
                    x1c_t = [fb.tile([128, D], F32, tag=f"x1c{i}", bufs=2,
                                     name=f"x1c{i}") for i in range(4)]
                    h2T_t = [fb.tile([128, T], BF16, tag=f"h2T{k}", bufs=2,
                                     name=f"h2T{k}") for k in range(KD)]
                    for i in range(4):
                        t = cc * 4 + i
                        nc.sync.dma_start(out=x1c_t[i], in_=x1d[t])
                        h2 = ftmp.tile([128, D], BF16, tag="h2")
                        nc.scalar.activation(out=h2, in_=x1c_t[i], func=AF.Identity,
                                             bias=nmr2[:, t:t + 1],
                                             scale=rstd2[:, t:t + 1])
                        for k in range(KD):
                            tp = ps_t.tile([128, 128], BF16, tag="tp")
                            nc.tensor.transpose(tp, h2[:, k * 128:(k + 1) * 128], ident)
                            nc.vector.tensor_copy(out=h2T_t[k][:, i * 128:(i + 1) * 128],
                                                  in_=tp)
                    return x1c_t, h2T_t

                x1c, h2T = ffn_chunk_in(0)
                for c in range(NB):  # 512-token chunks
                    G = [fb.tile([128, 2, T], F8, tag=f"G{f}", bufs=1, name=f"G{f}")
                         for f in range(KF2)]

                    # up-projection (bf16) + exact gelu -> fp8 G pairs
                    for f in range(KF):
                        w1t = w1s.tile([128, D], BF16, tag="w1t")
                        nc.sync.dma_start(out=w1t, in_=w1f_ext[f, :, :])
                        if c == 0:
                            nc.sync.dma_start(out=w2p_sb[f // 2][:, f % 2, :],
                                              in_=w2p_ext[f // 2, :, f % 2, :])
                        h1 = ps_h1.tile([128, T], F32, tag="h1")
                        for k in range(KD):
                            nc.tensor.matmul(h1, lhsT=w1t[:, k * 128:(k + 1) * 128],
                                             rhs=h2T[k], start=(k == 0), stop=(k == KD - 1))
                        nc.scalar.activation(out=G[f // 2][:, f % 2, :], in_=h1, func=AF.Gelu,
                                             bias=b1col[:, f:f + 1])

                    # hoist the next chunk's load+normalize+transposes ahead
                    # of the down-projection (keeps the chunk boundary clear)
                    if c + 1 < NB:
                        x1c_next, h2T_next = ffn_chunk_in(c + 1)

                    # down-projection (fp8 DR) + residual (both 16x scaled)
                    for i in range(4):
                        t = c * 4 + i
                        oc = ftmp.tile([128, D], F32, tag="oc")
                        for h in range(2):
                            sl = slice(h * 512, (h + 1) * 512)
                            ops = ps_o.tile([128, 512], F32, tag="o")
                            for f in range(KF2):
                                nc.tensor.matmul(ops,
                                                 lhsT=G[f][:, :, i * 128:(i + 1) * 128],
                                                 rhs=w2p_sb[f][:, :, sl], perf_mode=DRM,
                                                 start=(f == 0),
                                                 stop=(not with_bias and f == KF2 - 1))
                            if with_bias:
                                nc.tensor.matmul(ops, lhsT=ones1, rhs=brows[3][:, sl],
                                                 start=False, stop=True,
                                                 skip_group_check=True)
                            nc.vector.tensor_tensor(out=oc[:, sl], in0=ops,
                                                    in1=x1c[i][:, sl], op=ALU.add)
                            nc.sync.dma_start(out=out_ext[t * 128:(t + 1) * 128, sl],
                                              in_=oc[:, sl])
                    if c + 1 < NB:
                        x1c, h2T = x1c_next, h2T_next

    nc.compile()
    return nc


_CACHE = {}


def _prep_inputs(x, gamma, beta, wq, bq, wk, bk, wv, bv, wp, bp, wbias, w1, b1, w2, b2):
    bf = ml_dtypes.bfloat16
    e4 = ml_dtypes.float8_e4m3
    f32 = np.float32
    gamma = np.asarray(gamma, f32)
    beta = np.asarray(beta, f32)
    wq = np.asarray(wq, f32); wk = np.asarray(wk, f32)
    wv = np.asarray(wv, f32); wp = np.asarray(wp, f32)
    w1 = np.asarray(w1, f32); w2 = np.asarray(w2, f32)

    def pair8(w, s):
        # [D, N] -> [KD2, 128, 2, N] fp8, pairing adjacent 128-row blocks
        kk = w.shape[0] // 256
        return np.ascontiguousarray(
            (w * s).reshape(kk, 2, 128, w.shape[1]).transpose(0, 2, 1, 3)).astype(e4)

    wqp = pair8(gamma[:, None] * wq, SW)
    wkp = pair8(gamma[:, None] * wk, SW)
    wvp = pair8(gamma[:, None] * wv, SW)
    wpp = pair8(wp, SW)
    w2p = pair8(w2, SW2)
    w1_m = gamma[:, None] * w1

    bq_m = beta @ wq + np.asarray(bq, f32)
    bk_m = beta @ wk + np.asarray(bk, f32)
    bv_m = beta @ wv + np.asarray(bv, f32)
    b1_m = beta @ w1 + np.asarray(b1, f32)
    bp_m = np.asarray(bp, f32)
    b2_m = np.asarray(b2, f32)
    wbT = np.exp(np.asarray(wbias, f32)[:T, :T]).T.astype(bf)

    # w1 tiled for per-f streaming: w1f[f, p, k*128+c] = w1_m[k*128+p, f*128+c]
    w1f = np.ascontiguousarray(
        w1_m.reshape(KD, 128, KF, 128).transpose(2, 1, 0, 3).reshape(KF, 128, D)
    ).astype(bf)

    brow = np.stack([bk_m, bv_m, bp_m * SR, b2_m * SR]).astype(bf)      # [4, D]
    bqcol = np.ascontiguousarray(bq_m.reshape(KD, 128).T, f32)          # [128, KD]
    b1col = np.ascontiguousarray(b1_m.reshape(KF, 128).T, f32)          # [128, KF]

    with_bias = not (np.all(bk_m == 0) and np.all(bv_m == 0) and np.all(bp_m == 0)
                     and np.all(b2_m == 0))

    shared = dict(wqp=wqp, wkp=wkp, wvp=wvp, wpp=wpp, w1f=w1f, w2p=w2p,
                  wbT=np.ascontiguousarray(wbT), brow=brow, bqcol=bqcol, b1col=b1col)
    x = np.asarray(x, f32) * f32(SR)
    in_maps = []
    for core in range(NCORES):
        shard = np.ascontiguousarray(x[core * NB:(core + 1) * NB].reshape(NT, D))
        in_maps.append(dict(shared, x=shard))
    return in_maps, with_bias


def kernel(**inputs) -> np.ndarray:
    in_maps, with_bias = _prep_inputs(**inputs)
    key = ("nc", with_bias)
    if key not in _CACHE:
        _CACHE[key] = build_nc(with_bias)
    nc = _CACHE[key]
    res = run_bass_kernel_spmd(nc, in_maps, core_ids=list(range(NCORES)))
    out = np.empty((B, T, D), np.float32)
    inv = np.float32(1.0 / SR)
    for core in range(NCORES):
        out[core * NB:(core + 1) * NB] = (
            res.results[core]["out"].reshape(NB, T, D) * inv)
    return out



# revision 4
# speedup vs baseline: 1.1253x; 1.1253x over previous
"""AFT transformer block on 8 Trainium2 NeuronCores.

Data-parallel over batch: each core runs the full block for 4 of the 32
sequences (the AFT attention mixes only within a sequence, so no
collectives are needed).  Host side folds the shared LayerNorm affine
params into the GEMM weights, precomputes exp(wbias).T, and quantizes
all GEMM operands to fp8-e4m3 so every large matmul runs in DoubleRow
perf mode (two contraction rows per cell, ~2x tensor-engine
throughput): QKV, positional-bias, output projection, FFN up and FFN
down.

FFN accuracy trick (keeps fp8 under the error budget): split
  gelu(h) = h/2 + (h/2)*erf(h/sqrt2)
The linear h/2 part collapses through the down-projection into a
precomputed D x D matrix W12 = w1 @ w2 / 2 applied to the LN2 output in
bf16 (exact), so only the small nonlinear part S' = h*erf(h/sqrt2)
(RMS ~0.46x of gelu) goes through the fp8 down-projection.  This more
than halves the fp8 error of the FFN, paying for the fp8 up-projection.

Scaling scheme (keeps every fp8 operand in e4m3's normal range with no
extra de-scale ops on the device):
  x is pre-scaled by SR=16 on the host, so the residual stream is 16x.
  LN outputs stored at 1/4 scale, QKV/proj weights at 4 -> exact-scale
  psums.  wv additionally carries SY=4 so YT = sigmoid(Q)*num/den lands
  at 4x for fp8 storage; wp at 4 -> proj psum at 16x.  FFN: w1 at 4 ->
  h1 psum exact; S' unit-scale fp8; w2 at 8 and W12 at 32 -> down psum
  at 16x = 16*(0.5*S'@w2 + 0.5*h@w1@w2).  Host divides output by 16.
"""

import numpy as np
import ml_dtypes

import concourse.bass as bass
import concourse.mybir as mybir
import concourse.tile as tile
from concourse import bacc
from concourse.bass_utils import run_bass_kernel_spmd
from concourse.masks import make_identity

F32 = mybir.dt.float32
U32 = mybir.dt.uint32
BF16 = mybir.dt.bfloat16
F8 = mybir.dt.float8e4
AF = mybir.ActivationFunctionType
ALU = mybir.AluOpType
DRM = mybir.MatmulPerfMode.DoubleRow

B, T, D, FF = 32, 512, 1024, 4096
NCORES = 8
NB = B // NCORES          # sequences per core (4)
NT = NB * T               # tokens per core (2048)
KD = D // 128             # 8
KD2 = KD // 2             # 4 doublerow pairs over D
KF = FF // 128            # 32
KF2 = KF // 2             # 16 doublerow pairs over FF
TT = NT // 128            # 16 token tiles per core
EPS = 1e-5

SR = 16.0                 # residual-stream scale (x pre-scaled on host)
SH = 0.25                 # stored LN1-output scale
SW = 4.0                  # qkv/proj weight scale
SY = 4.0                  # stored Y^T scale (folded into wv)
SH2 = 0.25                # stored LN2-output scale
SW1 = 4.0                 # w1 weight scale (SH2*SW1 = 1 -> exact h1 psum)
SW2 = 8.0                 # w2 weight scale (= SR/2: down psum lands at 16x)
RSQ2 = 0.70710678118      # 1/sqrt(2) for the erf argument
EPS_S = EPS * SR * SR


def build_nc(with_bias: bool):
    nc = bacc.Bacc("TRN2", target_bir_lowering=False, debug=False, num_devices=NCORES)

    x_ext = nc.dram_tensor("x", [NT, D], F32, kind="ExternalInput").ap()
    # fp8 doublerow-paired weights: [pair, partition(k-within), 2, cols]
    wqp_ext = nc.dram_tensor("wqp", [KD2, 128, 2, D], F8, kind="ExternalInput").ap()
    wkp_ext = nc.dram_tensor("wkp", [KD2, 128, 2, D], F8, kind="ExternalInput").ap()
    wvp_ext = nc.dram_tensor("wvp", [KD2, 128, 2, D], F8, kind="ExternalInput").ap()
    wpp_ext = nc.dram_tensor("wpp", [KD2, 128, 2, D], F8, kind="ExternalInput").ap()
    w1p_ext = nc.dram_tensor("w1p", [KD2, 128, 2, FF], F8, kind="ExternalInput").ap()
    w2p_ext = nc.dram_tensor("w2p", [KF2, 128, 2, D], F8, kind="ExternalInput").ap()
    # positional-bias weights exp(wbias).T, fp8, paired over source blocks
    wbp_ext = nc.dram_tensor("wbp", [2, 128, 2, T], F8, kind="ExternalInput").ap()
    # FFN linear-path weights W12 = 32 * (w1' @ w2), bf16 row blocks
    W12_ext = nc.dram_tensor("W12", [KD, 128, D], BF16, kind="ExternalInput").ap()
    # rows: 0=bk', 1=bv'*SY, 2=bp'*SR, 3=(b2'+0.5*b1'@w2)*SR
    brow_ext = nc.dram_tensor("brow", [4, D], BF16, kind="ExternalInput").ap()
    b1row_ext = nc.dram_tensor("b1row", [1, FF], BF16, kind="ExternalInput").ap()
    bqcol_ext = nc.dram_tensor("bqcol", [128, KD], F32, kind="ExternalInput").ap()
    out_ext = nc.dram_tensor("out", [NT, D], F32, kind="ExternalOutput").ap()

    with tile.TileContext(nc) as tc:
        with (
            tc.tile_pool(name="const", bufs=1) as constp,
            tc.tile_pool(name="dstats", bufs=1) as dstats,
            tc.tile_pool(name="fw", bufs=1) as fw,
            tc.tile_pool(name="ps_t", bufs=2, space="PSUM") as ps_t,
            tc.tile_pool(name="x1dp", bufs=1, space="DRAM") as x1dp,
        ):
            ident = constp.tile([128, 128], BF16, tag="ident")
            make_identity(nc, ident)
            ones1 = constp.tile([1, T], BF16, tag="ones1")
            nc.vector.memset(ones1, 1.0)
            wbp_sb = constp.tile([128, 2, 2, T], F8, tag="wbp")
            brows = []
            for r in range(4):
                br = constp.tile([1, D], BF16, tag=f"brow{r}", name=f"brow{r}")
                nc.sync.dma_start(out=br, in_=brow_ext[r:r + 1, :])
                brows.append(br)
            b1row = constp.tile([1, FF], BF16, tag="b1row")
            nc.sync.dma_start(out=b1row, in_=b1row_ext[0:1, :])
            bqcol = constp.tile([128, KD], F32, tag="bqcol")
            nc.sync.dma_start(out=bqcol, in_=bqcol_ext[:, :])
            # warm the exp table set while the first activation DMAs land
            wt = constp.tile([128, 1], F32, tag="wt")
            nc.vector.memset(wt, 0.0)
            nc.scalar.activation(out=wt, in_=wt, func=AF.Exp)

            # FFN weights prefetched during the last attention batch
            w1p_sb = [fw.tile([128, 2, FF], F8, tag=f"w1_{k}", name=f"w1_{k}")
                      for k in range(KD2)]
            W12_sb = [fw.tile([128, D], BF16, tag=f"W12_{k}", name=f"W12_{k}")
                      for k in range(KD)]

            # per-token-tile LayerNorm stats, [128, tt] (one column per tile)
            mean1 = dstats.tile([128, TT], F32, tag="mean1")
            var1 = dstats.tile([128, TT], F32, tag="var1")
            rstd1 = dstats.tile([128, TT], F32, tag="rstd1")
            nmr1 = dstats.tile([128, TT], F32, tag="nmr1")
            mean2 = dstats.tile([128, TT], F32, tag="mean2")
            var2 = dstats.tile([128, TT], F32, tag="var2")
            rstd2 = dstats.tile([128, TT], F32, tag="rstd2")
            nmr2 = dstats.tile([128, TT], F32, tag="nmr2")
            nve = dstats.tile([128, 4], F32, tag="nve")   # rsqrt Newton scratch
            nt = dstats.tile([128, 4], F32, tag="nt")

            def rsqrt_dve(y, nmr_ap, mean_ap, var_ap, n, out_scale):
                """y = out_scale/sqrt(var+EPS_S), nmr = -mean*y (DVE only).
                y/nmr_ap/mean_ap/var_ap are pre-sliced [128, n] APs."""
                ve = nve[:, 0:n]
                t = nt[:, 0:n]
                nc.vector.tensor_scalar_add(out=ve, in0=var_ap, scalar1=EPS_S)
                nc.vector.tensor_scalar(out=y.bitcast(U32), in0=ve.bitcast(U32),
                                        scalar1=1, scalar2=None,
                                        op0=ALU.logical_shift_right)
                nc.vector.tensor_scalar(out=y.bitcast(U32), in0=y.bitcast(U32),
                                        scalar1=0x5f3759df, scalar2=-1,
                                        op0=ALU.subtract, op1=ALU.mult)
                for it in range(3):
                    s = out_scale if it == 2 else 1.0
                    nc.vector.tensor_tensor(out=t, in0=ve, in1=y, op=ALU.mult)
                    nc.vector.tensor_tensor(out=t, in0=t, in1=y, op=ALU.mult)
                    nc.vector.tensor_scalar(out=t, in0=t, scalar1=-0.5 * s,
                                            scalar2=1.5 * s,
                                            op0=ALU.mult, op1=ALU.add)
                    nc.vector.tensor_tensor(out=y, in0=y, in1=t, op=ALU.mult)
                nc.vector.tensor_tensor(out=nmr_ap, in0=mean_ap, in1=y, op=ALU.mult)
                nc.vector.tensor_scalar_mul(out=nmr_ap, in0=nmr_ap, scalar1=-1.0)

            # DRAM scratch for the post-attention residual stream x1 (16x)
            x1d = [x1dp.tile([128, D], F32, tag=f"x1d{t}", name=f"x1d{t}") for t in range(TT)]

            # ---------------- attention sub-block ----------------
            with (
                tc.tile_pool(name="aw", bufs=1) as aw,
                tc.tile_pool(name="xbp", bufs=2) as xbp,
                tc.tile_pool(name="ps_mm", bufs=6, space="PSUM") as ps_mm,
            ):
                xb0 = [xbp.tile([128, D], F32, tag=f"xb{i}", name=f"xb{i}") for i in range(4)]
                for i in range(4):
                    nc.sync.dma_start(out=xb0[i], in_=x_ext[i * 128:(i + 1) * 128, :])

                def ln_stats(xt, mean_t, var_t, t, stats_pool):
                    st = stats_pool.tile([128, 2, 6], F32, tag="bnst", name="st")
                    nc.vector.bn_stats(out=st[:, 0, :], in_=xt[:, 0:512])
                    nc.vector.bn_stats(out=st[:, 1, :], in_=xt[:, 512:1024])
                    mvt = stats_pool.tile([128, 2], F32, tag="mvt", name="mvt")
                    nc.vector.bn_aggr(out=mvt, in_=st)
                    nc.gpsimd.tensor_copy(out=mean_t[:, t:t + 1], in_=mvt[:, 0:1])
                    nc.gpsimd.tensor_copy(out=var_t[:, t:t + 1], in_=mvt[:, 1:2])
                    return mvt

                wqp_sb = [aw.tile([128, 2, D], F8, tag=f"wq{k}", name=f"wq{k}") for k in range(KD2)]
                wkp_sb = [aw.tile([128, 2, D], F8, tag=f"wk{k}", name=f"wk{k}") for k in range(KD2)]
                wvp_sb = [aw.tile([128, 2, D], F8, tag=f"wv{k}", name=f"wv{k}") for k in range(KD2)]
                wpp_sb = [aw.tile([128, 2, D], F8, tag=f"wp{k}", name=f"wp{k}") for k in range(KD2)]
                for k in range(KD2):
                    nc.sync.dma_start(out=wqp_sb[k], in_=wqp_ext[k])
                for k in range(KD2):
                    nc.sync.dma_start(out=wkp_sb[k], in_=wkp_ext[k])
                    nc.sync.dma_start(out=wvp_sb[k], in_=wvp_ext[k])
                for p in range(2):
                    nc.sync.dma_start(out=wbp_sb[:, p], in_=wbp_ext[p])
                for k in range(KD2):
                    nc.sync.dma_start(out=wpp_sb[k], in_=wpp_ext[k])

                with (
                    tc.tile_pool(name="ab", bufs=1) as ab,
                    tc.tile_pool(name="tmp", bufs=2) as tmp,
                    tc.tile_pool(name="sip", bufs=3) as sip,
                ):
                    # tile 0 fast path: rsqrt straight from bn_aggr output,
                    # skipping the gpsimd scatter hop (shorter startup chain)
                    mvt0 = ln_stats(xb0[0], mean1, var1, 0, sip)
                    rsqrt_dve(rstd1[:, 0:1], nmr1[:, 0:1],
                              mvt0[:, 0:1], mvt0[:, 1:2], 1, SH)
                    for t in range(1, 4):
                        ln_stats(xb0[t], mean1, var1, t, sip)
                    rsqrt_dve(rstd1[:, 1:4], nmr1[:, 1:4],
                              mean1[:, 1:4], var1[:, 1:4], 3, SH)

                    def norm_transpose(xb_t, col, h0T_t, i):
                        """h0 = SH*LN(x) (bf16) -> PE transpose -> fp8 pairs."""
                        h0 = tmp.tile([128, D], BF16, tag="h0", bufs=3)
                        nc.scalar.activation(out=h0, in_=xb_t, func=AF.Identity,
                                             bias=nmr1[:, col:col + 1],
                                             scale=rstd1[:, col:col + 1])
                        for k in range(KD):
                            tp = ps_t.tile([128, 128], BF16, tag="tp")
                            nc.tensor.transpose(tp, h0[:, k * 128:(k + 1) * 128], ident)
                            nc.vector.tensor_copy(
                                out=h0T_t[k // 2][:, k % 2, i * 128:(i + 1) * 128], in_=tp)

                    xb = xb0
                    x1_prev = None
                    # batch-0 transposes emitted ahead of the loop
                    h0T = [ab.tile([128, 2, T], F8, tag=f"h0T{k}", bufs=2, name=f"h0T{k}")
                           for k in range(KD2)]
                    for i in range(4):
                        norm_transpose(xb0[i], i, h0T, i)
                    for b in range(NB):
                        if b + 1 < NB:
                            xb_next = [xbp.tile([128, D], F32, tag=f"xb{i}", name=f"xb{i}")
                                       for i in range(4)]
                            for i in range(4):
                                t = (b + 1) * 4 + i
                                nc.sync.dma_start(out=xb_next[i],
                                                  in_=x_ext[t * 128:(t + 1) * 128, :])
                        if b == NB - 1:
                            # prefetch the resident FFN weights while the last
                            # batch computes (hides the load under attention)
                            for k in range(KD2):
                                nc.sync.dma_start(out=w1p_sb[k], in_=w1p_ext[k])
                            for k in range(KD):
                                nc.sync.dma_start(out=W12_sb[k], in_=W12_ext[k])
                        eKp = [ab.tile([128, 2, D], F8, tag=f"eKp{p}", bufs=1, name=f"eKp{p}")
                               for p in range(2)]
                        EVp = [ab.tile([128, 2, D], F8, tag=f"EVp{p}", bufs=1, name=f"EVp{p}")
                               for p in range(2)]
                        sg = [ab.tile([128, T], BF16, tag=f"sg{j}", bufs=1, name=f"sg{j}")
                              for j in range(KD)]
                        YT = [ab.tile([128, 2, T], F8, tag=f"YT{j}", bufs=1, name=f"YT{j}")
                              for j in range(KD2)]
                        x1t = [ab.tile([128, D], F32, tag=f"x1t{i}", bufs=2, name=f"x1t{i}")
                               for i in range(4)]

                        # Q -> sg = sigmoid(Q + bq)
                        for j in range(KD):
                            qps = ps_mm.tile([128, T], F32, tag="mm")
                            for k in range(KD2):
                                nc.tensor.matmul(qps,
                                                 lhsT=wqp_sb[k][:, :, j * 128:(j + 1) * 128],
                                                 rhs=h0T[k], perf_mode=DRM,
                                                 start=(k == 0), stop=(k == KD2 - 1))
                            nc.scalar.activation(out=sg[j], in_=qps, func=AF.Sigmoid,
                                                 bias=bqcol[:, j:j + 1])
                            # previous batch's LN2 stats ride the Q section's
                            # DVE slack (keeps the batch boundary clear)
                            if x1_prev is not None and j < 4:
                                ln_stats(x1_prev[j], mean2, var2,
                                         (b - 1) * 4 + j, sip)
                        if x1_prev is not None:
                            lo = (b - 1) * 4
                            rsqrt_dve(rstd2[:, lo:lo + 4], nmr2[:, lo:lo + 4],
                                      mean2[:, lo:lo + 4], var2[:, lo:lo + 4],
                                      4, SH2)

                        # K, V (token-major) -> exp(K), SY*exp(K)*V, fp8 pairs
                        for i in range(4):
                            for h in range(2):
                                sl = slice(h * 512, (h + 1) * 512)
                                kps = ps_mm.tile([128, 512], F32, tag="mm")
                                for k in range(KD2):
                                    nc.tensor.matmul(kps,
                                                     lhsT=h0T[k][:, :, i * 128:(i + 1) * 128],
                                                     rhs=wkp_sb[k][:, :, sl], perf_mode=DRM,
                                                     start=(k == 0),
                                                     stop=(not with_bias and k == KD2 - 1))
                                if with_bias:
                                    nc.tensor.matmul(kps, lhsT=ones1[:, 0:128],
                                                     rhs=brows[0][:, sl],
                                                     start=False, stop=True,
                                                     skip_group_check=True)
                                nc.scalar.activation(out=eKp[i // 2][:, i % 2, sl],
                                                     in_=kps, func=AF.Exp)
                                vps = ps_mm.tile([128, 512], F32, tag="mm")
                                for k in range(KD2):
                                    nc.tensor.matmul(vps,
                                                     lhsT=h0T[k][:, :, i * 128:(i + 1) * 128],
                                                     rhs=wvp_sb[k][:, :, sl], perf_mode=DRM,
                                                     start=(k == 0),
                                                     stop=(not with_bias and k == KD2 - 1))
                                if with_bias:
                                    nc.tensor.matmul(vps, lhsT=ones1[:, 0:128],
                                                     rhs=brows[1][:, sl],
                                                     start=False, stop=True,
                                                     skip_group_check=True)
                                nc.vector.tensor_tensor(out=EVp[i // 2][:, i % 2, sl],
                                                        in0=eKp[i // 2][:, i % 2, sl],
                                                        in1=vps, op=ALU.mult)
                            if b + 1 < NB:
                                ln_stats(xb_next[i], mean1, var1, (b + 1) * 4 + i, sip)
                        if b + 1 < NB:
                            lo = (b + 1) * 4
                            rsqrt_dve(rstd1[:, lo:lo + 4], nmr1[:, lo:lo + 4],
                                      mean1[:, lo:lo + 4], var1[:, lo:lo + 4], 4, SH)

                        # positional-bias matmuls (fp8 doublerow) + Y epilogue
                        # YT = sg * (SY*num) / den
                        for j in range(KD):
                            jsl = slice(j * 128, (j + 1) * 128)
                            dps = ps_mm.tile([128, T], F32, tag="mm")
                            for p in range(2):
                                nc.tensor.matmul(dps, lhsT=eKp[p][:, :, jsl],
                                                 rhs=wbp_sb[:, p], perf_mode=DRM,
                                                 start=(p == 0), stop=(p == 1))
                            nps = ps_mm.tile([128, T], F32, tag="mm")
                            for p in range(2):
                                nc.tensor.matmul(nps, lhsT=EVp[p][:, :, jsl],
                                                 rhs=wbp_sb[:, p], perf_mode=DRM,
                                                 start=(p == 0), stop=(p == 1))
                            rd = tmp.tile([128, T], F32, tag="rd")
                            nc.vector.reciprocal_approx_fast(out=rd, in_=dps)
                            t1 = tmp.tile([128, T], F32, tag="t1")
                            nc.vector.tensor_tensor(out=t1, in0=nps, in1=rd, op=ALU.mult)
                            nc.vector.tensor_tensor(out=YT[j // 2][:, j % 2, :], in0=t1,
                                                    in1=sg[j], op=ALU.mult)

                        # hoist the NEXT batch's normalize+transposes ahead of
                        # the projection so neither engine queue blocks at the
                        # batch boundary
                        if b + 1 < NB:
                            h0T_next = [ab.tile([128, 2, T], F8, tag=f"h0T{k}",
                                                bufs=2, name=f"h0T{k}")
                                        for k in range(KD2)]
                            for i in range(4):
                                norm_transpose(xb_next[i], (b + 1) * 4 + i,
                                               h0T_next, i)

                        # output projection (fp8 DR) + residual -> x1, LN2 stats
                        for i in range(4):
                            t = b * 4 + i
                            for h in range(2):
                                sl = slice(h * 512, (h + 1) * 512)
                                pps = ps_mm.tile([128, 512], F32, tag="mm")
                                for j in range(KD2):
                                    nc.tensor.matmul(pps,
                                                     lhsT=YT[j][:, :, i * 128:(i + 1) * 128],
                                                     rhs=wpp_sb[j][:, :, sl], perf_mode=DRM,
                                                     start=(j == 0),
                                                     stop=(not with_bias and j == KD2 - 1))
                                if with_bias:
                                    nc.tensor.matmul(pps, lhsT=ones1[:, 0:128],
                                                     rhs=brows[2][:, sl],
                                                     start=False, stop=True,
                                                     skip_group_check=True)
                                nc.vector.tensor_tensor(out=x1t[i][:, sl], in0=pps,
                                                        in1=xb[i][:, sl], op=ALU.add)
                            nc.sync.dma_start(out=x1d[t], in_=x1t[i])

                        if b == NB - 1:
                            # last batch: nobody left to defer to
                            for i in range(4):
                                ln_stats(x1t[i], mean2, var2, b * 4 + i, sip)
                            lo = b * 4
                            rsqrt_dve(rstd2[:, lo:lo + 4], nmr2[:, lo:lo + 4],
                                      mean2[:, lo:lo + 4], var2[:, lo:lo + 4],
                                      4, SH2)
                        x1_prev = x1t
                        if b + 1 < NB:
                            xb = xb_next
                            h0T = h0T_next

            # ---------------- FFN sub-block ----------------
            # gelu split: psum = 16*(0.5*S'@w2 + x_hat@W12/32*...) ; S' fp8 DR,
            # linear path bf16 via W12.  Up-projection fp8 DR from h2T pairs.
            with (
                tc.tile_pool(name="fw2", bufs=1) as fw2,
                tc.tile_pool(name="fb", bufs=1) as fb,
                tc.tile_pool(name="ftmp", bufs=3) as ftmp,
                tc.tile_pool(name="ps_o", bufs=3, space="PSUM") as ps_o,
                tc.tile_pool(name="ps_h1", bufs=2, space="PSUM") as ps_h1,
            ):
                w2p_sb = [fw2.tile([128, 2, D], F8, tag=f"w2_{f}", name=f"w2_{f}")
                          for f in range(KF2)]
                def ffn_chunk_in(cc):
                    """Load + normalize + transpose one 512-token chunk.
                    h2 = SH2*LN(x1) -> h2T bf16 (W12 path) + h2Tp fp8 pairs."""
                    x1c_t = [fb.tile([128, D], F32, tag=f"x1c{i}", bufs=2,
                                     name=f"x1c{i}") for i in range(4)]
                    h2T_t = [fb.tile([128, T], BF16, tag=f"h2T{k}", bufs=2,
                                     name=f"h2T{k}") for k in range(KD)]
                    h2Tp_t = [fb.tile([128, 2, T], F8, tag=f"h2Tp{k}", bufs=2,
                                      name=f"h2Tp{k}") for k in range(KD2)]
                    for i in range(4):
                        t = cc * 4 + i
                        nc.sync.dma_start(out=x1c_t[i], in_=x1d[t])
                        h2 = ftmp.tile([128, D], BF16, tag="h2")
                        nc.scalar.activation(out=h2, in_=x1c_t[i], func=AF.Identity,
                                             bias=nmr2[:, t:t + 1],
                                             scale=rstd2[:, t:t + 1])
                        for k in range(KD):
                            tp = ps_t.tile([128, 128], BF16, tag="tp")
                            nc.tensor.transpose(tp, h2[:, k * 128:(k + 1) * 128], ident)
                            isl = slice(i * 128, (i + 1) * 128)
                            nc.vector.tensor_copy(out=h2T_t[k][:, isl], in_=tp)
                            nc.vector.tensor_copy(out=h2Tp_t[k // 2][:, k % 2, isl],
                                                  in_=tp)
                    return x1c_t, h2T_t, h2Tp_t

                x1c, h2T, h2Tp = ffn_chunk_in(0)
                for c in range(NB):  # 512-token chunks
                    Sp = [fb.tile([128, 2, T], F8, tag=f"Sp{f}", bufs=1, name=f"Sp{f}")
                          for f in range(KF2)]

                    # up-projection (fp8 DR) -> E = erf(h1/sqrt2), S' = E*h1
                    for f in range(KF):
                        if c == 0:
                            nc.sync.dma_start(out=w2p_sb[f // 2][:, f % 2, :],
                                              in_=w2p_ext[f // 2, :, f % 2, :])
                        h1 = ps_h1.tile([128, T], F32, tag="h1")
                        fsl = slice(f * 128, (f + 1) * 128)
                        for k in range(KD2):
                            nc.tensor.matmul(h1, lhsT=w1p_sb[k][:, :, fsl],
                                             rhs=h2Tp[k], perf_mode=DRM,
                                             start=(k == 0),
                                             stop=(not with_bias and k == KD2 - 1))
                        if with_bias:
                            # per-partition bias: rank-1 with b1 row as lhsT
                            nc.tensor.matmul(h1, lhsT=b1row[:, fsl], rhs=ones1,
                                             start=False, stop=True,
                                             skip_group_check=True)
                        E = ftmp.tile([128, T], BF16, tag="E")
                        nc.scalar.activation(out=E, in_=h1, func=AF.Erf, scale=RSQ2)
                        nc.vector.tensor_tensor(out=Sp[f // 2][:, f % 2, :],
                                                in0=h1, in1=E, op=ALU.mult)

                    # hoist the next chunk's load+normalize+transposes ahead
                    # of the down-projection (keeps the chunk boundary clear)
                    if c + 1 < NB:
                        x1c_next, h2T_next, h2Tp_next = ffn_chunk_in(c + 1)

                    # down-projection: fp8 DR on S' + bf16 W12 linear path,
                    # both accumulating into the same psum, + residual (16x)
                    for i in range(4):
                        t = c * 4 + i
                        isl = slice(i * 128, (i + 1) * 128)
                        oc = ftmp.tile([128, D], F32, tag="oc")
                        for h in range(2):
                            sl = slice(h * 512, (h + 1) * 512)
                            ops = ps_o.tile([128, 512], F32, tag="o")
                            for f in range(KF2):
                                nc.tensor.matmul(ops,
                                                 lhsT=Sp[f][:, :, isl],
                                                 rhs=w2p_sb[f][:, :, sl], perf_mode=DRM,
                                                 start=(f == 0), stop=False)
                            for k in range(KD):
                                nc.tensor.matmul(ops, lhsT=h2T[k][:, isl],
                                                 rhs=W12_sb[k][:, sl],
                                                 start=False,
                                                 stop=(not with_bias and k == KD - 1),
                                                 skip_group_check=True)
                            if with_bias:
                                nc.tensor.matmul(ops, lhsT=ones1[:, 0:128],
                                                 rhs=brows[3][:, sl],
                                                 start=False, stop=True,
                                                 skip_group_check=True)
                            nc.vector.tensor_tensor(out=oc[:, sl], in0=ops,
                                                    in1=x1c[i][:, sl], op=ALU.add)
                            nc.sync.dma_start(out=out_ext[t * 128:(t + 1) * 128, sl],
                                              in_=oc[:, sl])
                    if c + 1 < NB:
                        x1c, h2T, h2Tp = x1c_next, h2T_next, h2Tp_next

    nc.compile()
    return nc


_CACHE = {}


def _prep_inputs(x, gamma, beta, wq, bq, wk, bk, wv, bv, wp, bp, wbias, w1, b1, w2, b2):
    bf = ml_dtypes.bfloat16
    e4 = ml_dtypes.float8_e4m3
    f32 = np.float32
    gamma = np.asarray(gamma, f32)
    beta = np.asarray(beta, f32)
    wq = np.asarray(wq, f32); wk = np.asarray(wk, f32)
    wv = np.asarray(wv, f32); wp = np.asarray(wp, f32)
    w1 = np.asarray(w1, f32); w2 = np.asarray(w2, f32)

    def pair8(w, s):
        # [D, N] -> [KD2, 128, 2, N] fp8, pairing adjacent 128-row blocks
        kk = w.shape[0] // 256
        return np.ascontiguousarray(
            (w * s).reshape(kk, 2, 128, w.shape[1]).transpose(0, 2, 1, 3)).astype(e4)

    w1_m = gamma[:, None] * w1
    wqp = pair8(gamma[:, None] * wq, SW)
    wkp = pair8(gamma[:, None] * wk, SW)
    wvp = pair8(gamma[:, None] * wv, SW * SY)
    wpp = pair8(wp, SW)
    w1p = pair8(w1_m, SW1)
    w2p = pair8(w2, SW2)
    # W12 linear path: psum += (SH2*xhat)@W12 must equal SR*0.5*xhat@(w1'@w2)
    W12 = np.ascontiguousarray(
        ((SR * 0.5 / SH2) * (w1_m @ w2)).reshape(KD, 128, D)).astype(bf)

    bq_m = beta @ wq + np.asarray(bq, f32)
    bk_m = beta @ wk + np.asarray(bk, f32)
    bv_m = beta @ wv + np.asarray(bv, f32)
    b1_m = beta @ w1 + np.asarray(b1, f32)
    bp_m = np.asarray(bp, f32)
    b2_m = np.asarray(b2, f32) + 0.5 * (b1_m @ w2)
    wbT = np.exp(np.asarray(wbias, f32)[:T, :T]).T
    wbp = np.ascontiguousarray(
        wbT.reshape(2, 2, 128, T).transpose(0, 2, 1, 3)).astype(e4)

    brow = np.stack([bk_m, bv_m * SY, bp_m * SR, b2_m * SR]).astype(bf)     # [4, D]
    b1row = np.ascontiguousarray(b1_m.reshape(1, FF)).astype(bf)
    bqcol = np.ascontiguousarray(bq_m.reshape(KD, 128).T, f32)              # [128, KD]

    with_bias = not (np.all(bk_m == 0) and np.all(bv_m == 0) and np.all(bp_m == 0)
                     and np.all(b2_m == 0) and np.all(b1_m == 0))

    shared = dict(wqp=wqp, wkp=wkp, wvp=wvp, wpp=wpp, w1p=w1p, w2p=w2p,
                  W12=W12, wbp=wbp, brow=brow, b1row=b1row, bqcol=bqcol)
    x = np.asarray(x, f32) * f32(SR)
    in_maps = []
    for core in range(NCORES):
        shard = np.ascontiguousarray(x[core * NB:(core + 1) * NB].reshape(NT, D))
        in_maps.append(dict(shared, x=shard))
    return in_maps, with_bias


def kernel(**inputs) -> np.ndarray:
    in_maps, with_bias = _prep_inputs(**inputs)
    key = ("nc", with_bias)
    if key not in _CACHE:
        _CACHE[key] = build_nc(with_bias)
    nc = _CACHE[key]
    res = run_bass_kernel_spmd(nc, in_maps, core_ids=list(range(NCORES)))
    out = np.empty((B, T, D), np.float32)
    inv = np.float32(1.0 / SR)
    for core in range(NCORES):
        out[core * NB:(core + 1) * NB] = (
            res.results[core]["out"].reshape(NB, T, D) * inv)
    return out
